# revision 1
# baseline (speedup 1.0000x reference)
"""Contrastive loss kernel for Trainium2 (8 NeuronCores, SPMD).

Math: loss = mean_{pos pairs}(1-cos_sim)^2 + mean_{neg pairs}relu(cos_sim-1)^2
with pos = same-label upper-triangle pairs, neg = different-label ordered pairs.

Strategy:
  * Host sorts rows by label so same-label pairs form a narrow diagonal band,
    and rotates columns per core so the band lands at the same local columns
    on every core (one uniform SPMD program).
  * Each core computes its [512, 4096] slice of the Gram matrix in bf16 on
    the PE (raw, unnormalized rhs; lhsT pre-scaled by 1/norm).
  * Norms come from a row-major squared-sum pipeline (ScalarE activation
    accumulate), inverted on VectorE in a compact [128, 32] layout, and
    broadcast along partitions via a K=1 ones-matmul.
  * Epilogue per PSUM tile: multiply by inv_j (column side of normalization),
    relu(s-1) then Square-accumulate => neg partials over ALL pairs; on the
    diagonal band only, index masks (computed from targets-derived per-row
    bounds) give the pos partials and a same-label correction to subtract
    from the neg sum.
  * Exact pair counts are integer combinatorics of targets, done on host.
    Host combines 8 x [128, 16] partial-stat tensors into the final scalar.
"""

import numpy as np
import ml_dtypes

import concourse.bass as bass
import concourse.bacc as bacc
import concourse.mybir as mybir
import concourse.tile as tile

N, D, NCORES = 4096, 512, 8
RPC = N // NCORES  # 512 rows per core
BAND_W = 512       # band slice width (covers all same-label cols per strip)
BMAX = 192         # max same-label block size the fixed band supports

F32 = mybir.dt.float32
BF16 = mybir.dt.bfloat16
AF = mybir.ActivationFunctionType
ALU = mybir.AluOpType


def build_program():
    nc = bacc.Bacc(None)
    xt16_d = nc.declare_dram_parameter("xt16", [D, N], BF16, isOutput=False)
    xr16_d = nc.declare_dram_parameter("xr16", [D, N], BF16, isOutput=False)
    meta_d = nc.declare_dram_parameter("meta", [128, BAND_W + 16], F32,
                                       isOutput=False)
    stats_d = nc.declare_dram_parameter("stats", [128, 16], F32, isOutput=True)
    scratch = nc.dram_tensor("invbounce", [N], F32)

    with tile.TileContext(nc) as tc:
        with (
            tc.tile_pool(name="perm", bufs=1) as perm,
            tc.tile_pool(name="rows", bufs=4) as rows,
            tc.tile_pool(name="rjunk", bufs=2) as rjunk,
            tc.tile_pool(name="work", bufs=2) as work,
            tc.tile_pool(name="bandp", bufs=2) as bandp,
            tc.tile_pool(name="psum", bufs=2, space="PSUM") as psum,
        ):
            meta_t = perm.tile([128, BAND_W + 16], F32, tag="meta")
            nc.sync.dma_start(meta_t[:], meta_d[:])
            iota_t = meta_t[:, 0:BAND_W]
            aux_t = meta_t[:, BAND_W:BAND_W + 16]
            stats_t = perm.tile([128, 16], F32, tag="stats")
            sumsq = perm.tile([128, 32], F32, tag="sumsq")
            xt_c = [perm.tile([128, N], BF16, tag=f"xt{k}", name=f"xt{k}") for k in range(4)]
            invf = perm.tile([128, N], F32, tag="invf")
            inv16own = perm.tile([128, RPC], BF16, tag="inv16own")
            xtL = [perm.tile([128, RPC], BF16, tag=f"xtL{k}", name=f"xtL{k}") for k in range(4)]
            flatF = perm.tile([1, N], F32, tag="flatF")
            nrm = perm.tile([128, 32], F32, tag="nrm")
            nrmx = perm.tile([128, 32], F32, tag="nrmx")
            invr = perm.tile([128, 32], F32, tag="invr")

            # --- row-major norms pipeline (overlaps DMA) ---
            # slab g holds 8 row-tiles: partition p, cols [512t', 512t'+512)
            # = local column j = 32p + 8g + t'
            for g in range(4):
                rt = rows.tile([128, N], BF16, tag="rt", name=f"rt{g}")
                for hh in range(2):
                    nc.sync.dma_start(
                        rt[:, 2048 * hh:2048 * (hh + 1)],
                        xr16_d[128 * g:128 * (g + 1),
                               2048 * hh:2048 * (hh + 1)])
                for tp in range(8):
                    t = 8 * g + tp
                    jk = rjunk.tile([128, D], BF16, tag="rj", name=f"rj{t}")
                    sl = rt[:, 512 * tp:512 * (tp + 1)]
                    nc.scalar.activation(jk[:], sl, AF.Square,
                                          accum_out=sumsq[:, t:t + 1])

            # --- transposed-chunk DMAs, first halves first ---
            for h in range(2):
                for k in range(4):
                    nc.sync.dma_start(
                        xt_c[k][:, 2048 * h:2048 * (h + 1)],
                        xt16_d[128 * k:128 * (k + 1), 2048 * h:2048 * (h + 1)])

            # --- inv = 1/max(sqrt(sumsq), eps), in compact layout ---
            nc.scalar.activation(nrm[:], sumsq[:], AF.Sqrt)
            nc.vector.tensor_scalar(out=nrmx[:], in0=nrm[:], scalar1=1e-8,
                                    scalar2=None, op0=ALU.max)
            nc.vector.reciprocal(invr[:], nrmx[:])

            # --- reorder [128,32] -> [1,4096] (partition-gather DMA) ---
            nc.sync.dma_start(
                flatF[0:1, :].rearrange("o (p t) -> o p t", p=128), invr[:])

            # --- broadcast inv along partitions via K=1 f32r matmul ---
            onesK = perm.tile([1, 128], F32, tag="onesK")
            nc.vector.memset(onesK[:], 1.0)

            def bcast_half(h):
                bmg = psum.tile([128, 2048], F32, tag="mega", name=f"bmg{h}")
                for t4 in range(4):
                    nc.tensor.matmul(
                        bmg[:, 512 * t4:512 * (t4 + 1)],
                        onesK[0:1, :].bitcast(mybir.dt.float32r),
                        flatF[0:1, 2048 * h + 512 * t4:
                              2048 * h + 512 * (t4 + 1)].bitcast(
                                  mybir.dt.float32r),
                        start=True, stop=True)
                nc.scalar.activation(invf[:, 2048 * h:2048 * (h + 1)], bmg[:],
                                     AF.Copy)

            # h=0 first: inv16own lives in [256,768) so xtL can start while
            # the h=1 broadcast/copy overlaps with early Gram work
            bcast_half(0)
            nc.scalar.activation(inv16own[:], invf[:, 256:768], AF.Copy)
            for k in range(4):
                eng = nc.vector if k % 2 == 0 else nc.gpsimd
                eng.tensor_tensor(xtL[k][:], xt_c[k][:, 256:768],
                                  inv16own[:], ALU.mult)
            bcast_half(1)

            # --- Gram megatiles + epilogue ---
            for h in range(2):
                for s in range(4):
                    mi = 4 * h + s
                    sim = psum.tile([128, 2048], F32, tag="mega")
                    for k in range(4):
                        for t4 in range(4):
                            nc.tensor.matmul(
                                sim[:, 512 * t4:512 * (t4 + 1)],
                                xtL[k][:, 128 * s:128 * (s + 1)],
                                xt_c[k][:, 2048 * h + 512 * t4:
                                          2048 * h + 512 * (t4 + 1)],
                                start=(k == 0), stop=(k == 3))
                    sb = work.tile([128, 2048], BF16, tag="sb")
                    nc.vector.tensor_tensor(sb[:], sim[:],
                                            invf[:, 2048 * h:2048 * (h + 1)],
                                            ALU.mult)
                    rb = work.tile([128, 2048], BF16, tag="rb")
                    nc.vector.tensor_scalar(out=rb[:], in0=sb[:], scalar1=1.0,
                                            scalar2=0.0, op0=ALU.subtract,
                                            op1=ALU.max)
                    jk2 = work.tile([128, 2048], BF16, tag="jk")
                    nc.scalar.activation(jk2[:], rb[:], AF.Square,
                                         accum_out=stats_t[:, mi:mi + 1])
                    if h == 0:
                        a = 64 + 128 * s
                        u1 = bandp.tile([128, BAND_W], BF16, tag="u1")
                        nc.vector.tensor_scalar(out=u1[:], in0=sb[:, a:a + BAND_W],
                                                scalar1=1.0, scalar2=None,
                                                op0=ALU.subtract)
                        chi = bandp.tile([128, BAND_W], BF16, tag="chi")
                        nc.vector.tensor_scalar(out=chi[:], in0=iota_t,
                                                scalar1=aux_t[:, 4 * s + 2:4 * s + 3],
                                                scalar2=None, op0=ALU.is_lt)
                        b1 = bandp.tile([128, BAND_W], BF16, tag="b1")
                        nc.vector.tensor_scalar(out=b1[:], in0=iota_t,
                                                scalar1=aux_t[:, 4 * s:4 * s + 1],
                                                scalar2=None, op0=ALU.is_gt)
                        a1 = bandp.tile([128, BAND_W], BF16, tag="a1")
                        nc.vector.tensor_scalar(out=a1[:], in0=iota_t,
                                                scalar1=aux_t[:, 4 * s + 1:4 * s + 2],
                                                scalar2=None, op0=ALU.is_ge)
                        pu = bandp.tile([128, BAND_W], BF16, tag="pu")
                        nc.gpsimd.tensor_tensor(pu[:], b1[:], chi[:], ALU.mult)
                        tm = bandp.tile([128, BAND_W], BF16, tag="tm")
                        nc.gpsimd.tensor_tensor(tm[:], a1[:], chi[:], ALU.mult)
                        v = bandp.tile([128, BAND_W], BF16, tag="v")
                        nc.gpsimd.tensor_tensor(v[:], u1[:], pu[:], ALU.mult)
                        g = bandp.tile([128, BAND_W], BF16, tag="g")
                        nc.gpsimd.tensor_tensor(g[:], rb[:, a:a + BAND_W],
                                                tm[:], ALU.mult)
                        bj1 = bandp.tile([128, BAND_W], BF16, tag="bj1")
                        nc.scalar.activation(bj1[:], v[:], AF.Square,
                                             accum_out=stats_t[:, 8 + s:9 + s])
                        bj2 = bandp.tile([128, BAND_W], BF16, tag="bj2")
                        nc.scalar.activation(bj2[:], g[:], AF.Square,
                                             accum_out=stats_t[:, 12 + s:13 + s])

            nc.sync.dma_start(stats_d[:], stats_t[:])
    nc.finalize()
    return nc


def host_prepare(inputs, targets):
    """Sort/rotate/pack per-core inputs. Returns (in_maps, counts)."""
    inputs = np.asarray(inputs, np.float32)
    targets_np = np.asarray(targets)
    order = np.argsort(targets_np, kind="stable")
    tss = targets_np[order]
    X = inputs[order]
    lo = np.searchsorted(tss, tss, side="left").astype(np.int64)
    hi = np.searchsorted(tss, tss, side="right").astype(np.int64)
    bmax = int((hi - lo).max())
    if bmax > BMAX:
        raise NotImplementedError(
            f"label block of size {bmax} exceeds supported band ({BMAX})")

    X16 = X.astype(ml_dtypes.bfloat16)
    # slab layout [512, 4096]: slab g partition p cols [512t',512t'+512) hold
    # local column j = 32p + 8g + t', so sumsq[p, 8g+t'] = sumsq_j with
    # j = 32p + (8g+t') and the [128,32] inv tile flattens linearly through
    # the DRAM bounce
    g_idx = np.arange(4)[:, None, None]          # slab
    p_idx = np.arange(128)[None, :, None]        # partition
    tp_idx = np.arange(8)[None, None, :]         # tile-in-slab
    j_map = (32 * p_idx + 8 * g_idx + tp_idx)    # [4, 128, 8]


    in_maps = []
    for c in range(NCORES):
        off = (RPC * c - 256) % N
        colmap = (np.arange(N) + off) % N  # local j -> global sorted row
        Xc = X16[colmap, :]
        xt16_c = np.ascontiguousarray(Xc.T)
        # [4, 128, 8, 512] -> [512, 4096]
        xr16_c = np.ascontiguousarray(
            Xc[j_map, :].reshape(4, 128, 8 * D).reshape(512, 4096))
        meta = np.zeros((128, BAND_W + 16), np.float32)
        meta[:, 0:BAND_W] = np.arange(BAND_W, dtype=np.float32)[None, :]
        aux = meta[:, BAND_W:BAND_W + 16]
        for s in range(4):
            a_s = 64 + 128 * s
            gi = RPC * c + 128 * s + np.arange(128)
            base = RPC * c - 256 + a_s
            i_cmp = (gi - base).astype(np.float32)
            lo_cmp = (lo[gi] - base).astype(np.float32)
            hi_cmp = (hi[gi] - base).astype(np.float32)
            assert (lo_cmp >= 0).all() and (hi_cmp <= BAND_W).all()
            aux[:, 4 * s + 0] = i_cmp
            aux[:, 4 * s + 1] = lo_cmp
            aux[:, 4 * s + 2] = hi_cmp
        in_maps.append({
            "xt16": xt16_c,
            "xr16": xr16_c,
            "meta": meta,
        })

    cnts = np.bincount(targets_np.astype(np.int64))
    pos_cnt = float((cnts * (cnts - 1) // 2).sum())
    neg_cnt = float(N * N - (cnts * cnts).sum())
    return in_maps, pos_cnt, neg_cnt


def combine(stats_list, pos_cnt, neg_cnt):
    neg_all = 0.0
    pos_sum = 0.0
    corr = 0.0
    for st in stats_list:
        st = np.asarray(st, np.float64)
        neg_all += st[:, 0:8].sum()
        pos_sum += st[:, 8:12].sum()
        corr += st[:, 12:16].sum()
    loss = np.float32(pos_sum / pos_cnt + (neg_all - corr) / neg_cnt)
    return np.asarray(loss, np.float32)


_prog_cache = {}


def kernel(inputs, targets):
    from concourse.bass_utils import run_bass_kernel_spmd
    in_maps, pos_cnt, neg_cnt = host_prepare(inputs, targets)
    if "nc" not in _prog_cache:
        _prog_cache["nc"] = build_program()
    nc = _prog_cache["nc"]
    res = run_bass_kernel_spmd(nc, in_maps, list(range(NCORES)))
    stats_list = [res.results[c]["stats"] for c in range(NCORES)]
    return combine(stats_list, pos_cnt, neg_cnt)



# revision 8
# speedup vs baseline: 3.3280x; 3.3280x over previous
"""Contrastive loss kernel for Trainium2 (8 NeuronCores, SPMD).

Math: loss = mean_{pos pairs}(1-cos_sim)^2 + mean_{neg pairs}relu(cos_sim-1)^2
with pos = same-label upper-triangle pairs, neg = different-label ordered pairs.

Since cosine similarity never exceeds 1 (up to ~1e-7 float rounding, squared
to ~1e-14), the neg term is identically zero and only the pos term is
computed. Host sorts rows by label so pos pairs form a narrow diagonal band
(max label-block size <= 129 supported), normalizes rows, and packs per-core
bf16 windows plus per-row band masks.

Each core c owns sorted rows [512c, 512c+512) and computes the [128, 256]
band Gram tile for each of its 4 row-strips against a 640-wide column window
(both sides of every matmul come from one window load). The epilogue per
strip: u = sim - 1 (VectorE, PSUM read), t = u * mask (GpSimd),
Square-accumulate (ScalarE) -> per-partition partial sums; pair count from
host-side integer combinatorics of targets.
"""

import numpy as np
import ml_dtypes

import concourse.bass as bass
import concourse.bacc as bacc
import concourse.mybir as mybir
import concourse.tile as tile

N, D, NCORES = 4096, 512, 8
RPC = N // NCORES   # 512 rows per core
WIN = 640           # column window width per core
BW = 256            # band width per 128-row strip
NSTRIP = RPC // 128
BMAX = 129          # max label-block size the band supports

F32 = mybir.dt.float32
BF16 = mybir.dt.bfloat16
AF = mybir.ActivationFunctionType
ALU = mybir.AluOpType
EPS = 1e-8


def build_program():
    nc = bacc.Bacc(None)
    xw_d = nc.declare_dram_parameter("xw", [D, WIN], BF16, isOutput=False)
    mk_d = nc.declare_dram_parameter("mk", [128, NSTRIP * BW], BF16,
                                     isOutput=False)
    stats_d = nc.declare_dram_parameter("stats", [128, NSTRIP], F32,
                                        isOutput=True)

    with tile.TileContext(nc) as tc:
        with (
            tc.tile_pool(name="perm", bufs=1) as perm,
            tc.tile_pool(name="work", bufs=2) as work,
            tc.tile_pool(name="psum", bufs=3, space="PSUM") as psum,
        ):
            xw_t = perm.tile([128, 4 * WIN], BF16, tag="xw")
            mk_t = perm.tile([128, NSTRIP * BW], BF16, tag="mk")
            statsA = perm.tile([128, NSTRIP], F32, tag="sa")

            # DMA schedule: strip-0 data first, then each later strip's
            # window piece + mask just ahead of its compute.
            for k in range(4):
                nc.sync.dma_start(xw_t[:, WIN * k:WIN * k + 256],
                                  xw_d[128 * k:128 * (k + 1), 0:256])
            nc.sync.dma_start(mk_t[:, 0:BW], mk_d[:, 0:BW])
            for k in range(4):
                nc.sync.dma_start(xw_t[:, WIN * k + 256:WIN * k + 448],
                                  xw_d[128 * k:128 * (k + 1), 256:448])
            nc.sync.dma_start(mk_t[:, BW:2 * BW], mk_d[:, BW:2 * BW])
            for k in range(4):
                nc.sync.dma_start(xw_t[:, WIN * k + 448:WIN * k + 640],
                                  xw_d[128 * k:128 * (k + 1), 448:640])
            nc.sync.dma_start(mk_t[:, 2 * BW:4 * BW], mk_d[:, 2 * BW:4 * BW])

            for s in range(NSTRIP):
                ps = psum.tile([128, BW], F32, tag="ps")
                for k in range(4):
                    a = WIN * k + 128 * s
                    nc.tensor.matmul(ps[:], xw_t[:, a:a + 128],
                                     xw_t[:, a:a + BW],
                                     start=(k == 0), stop=(k == 3))
                u = work.tile([128, BW], BF16, tag="u")
                nc.vector.tensor_scalar(out=u[:], in0=ps[:], scalar1=1.0,
                                        scalar2=None, op0=ALU.subtract)
                t = work.tile([128, BW], BF16, tag="t")
                nc.gpsimd.tensor_tensor(t[:], u[:],
                                        mk_t[:, BW * s:BW * (s + 1)],
                                        ALU.mult)
                jk = work.tile([128, BW], BF16, tag="jk")
                nc.scalar.activation(jk[:], t[:], AF.Square,
                                     accum_out=statsA[:, s:s + 1])

            nc.sync.dma_start(stats_d[:], statsA[:])
    nc.finalize()
    return nc


def host_prepare(inputs, targets):
    """Sort by label, normalize, pack per-core windows + band masks.

    Returns (in_maps, pos_cnt); in_maps is None if a label block exceeds
    the supported band (fallback to host compute).
    """
    X = np.asarray(inputs, np.float32)
    tg = np.asarray(targets)
    order = np.argsort(tg, kind="stable")
    tss = tg[order]
    Xs = X[order]
    lo = np.searchsorted(tss, tss, side="left")
    hi = np.searchsorted(tss, tss, side="right")
    cnts = np.bincount(tg.astype(np.int64))
    pos_cnt = float((cnts.astype(np.int64) * (cnts - 1) // 2).sum())
    if int((hi - lo).max()) > BMAX:
        return None, pos_cnt

    nrm = np.sqrt((Xs * Xs).sum(axis=1, keepdims=True))
    Xn = (Xs / np.maximum(nrm, EPS)).astype(ml_dtypes.bfloat16)

    p = np.arange(128)[:, None]
    b = np.arange(BW)[None, :]
    in_maps = []
    for c in range(NCORES):
        gidx = (RPC * c + np.arange(WIN)) % N
        xw = np.ascontiguousarray(Xn[gidx, :].T)
        mk = np.zeros((128, NSTRIP * BW), ml_dtypes.bfloat16)
        for s in range(NSTRIP):
            gi = RPC * c + 128 * s + np.arange(128)
            hi_cmp = (hi[gi] - (RPC * c + 128 * s))[:, None]
            assert (hi_cmp <= BW).all()
            mk[:, BW * s:BW * (s + 1)] = ((b > p) & (b < hi_cmp)).astype(
                ml_dtypes.bfloat16)
        in_maps.append({"xw": xw, "mk": mk})
    return in_maps, pos_cnt


def combine(stats_list, pos_cnt):
    a_sum = 0.0
    for st in stats_list:
        st = np.asarray(st, np.float64)
        a_sum += st[:, 0:NSTRIP].sum()
    return np.asarray(np.float32(a_sum / pos_cnt))


def _host_fallback(inputs, targets):
    X = np.asarray(inputs, np.float64)
    tg = np.asarray(targets)
    nrm = np.sqrt((X * X).sum(axis=1, keepdims=True))
    x = X / np.maximum(nrm, EPS)
    total = 0.0
    pos_cnt = 0
    for lbl in np.unique(tg):
        xl = x[tg == lbl]
        m = xl.shape[0]
        if m < 2:
            continue
        S = xl @ xl.T
        iu = np.triu_indices(m, k=1)
        total += ((1.0 - S[iu]) ** 2).sum()
        pos_cnt += m * (m - 1) // 2
    return np.asarray(np.float32(total / pos_cnt))


_prog_cache = {}


def kernel(inputs, targets):
    from concourse.bass_utils import run_bass_kernel_spmd
    in_maps, pos_cnt = host_prepare(inputs, targets)
    if in_maps is None:
        return _host_fallback(inputs, targets)
    if "nc" not in _prog_cache:
        _prog_cache["nc"] = build_program()
    nc = _prog_cache["nc"]
    res = run_bass_kernel_spmd(nc, in_maps, list(range(NCORES)))
    stats_list = [res.results[c]["stats"] for c in range(NCORES)]
    return combine(stats_list, pos_cnt)


# revision 11
# speedup vs baseline: 4.0922x; 1.2296x over previous
"""Contrastive loss kernel for Trainium2 (8 NeuronCores, SPMD).

Math: loss = mean_{pos pairs}(1-cos_sim)^2 + mean_{neg pairs}relu(cos_sim-1)^2
with pos = same-label upper-triangle pairs, neg = different-label ordered pairs.

Since cosine similarity never exceeds 1 (up to ~1e-7 float rounding, squared
to ~1e-14), the neg term is identically zero and only the pos term is
computed. Host sorts rows by label so pos pairs form a narrow diagonal band
(max label-block size <= 129 supported), normalizes rows, and packs per-core
bf16 windows plus per-row band masks.

Each core c owns sorted rows [512c, 512c+512) and computes the [128, 256]
band Gram tile for each of its 4 row-strips against a 640-wide column window
(both sides of every matmul come from one window load). The epilogue per
strip: u = sim - 1 (VectorE, PSUM read), t = u * mask (GpSimd),
Square-accumulate (ScalarE) -> per-partition partial sums; pair count from
host-side integer combinatorics of targets.
"""

import numpy as np
import ml_dtypes

import concourse.bass as bass
import concourse.bacc as bacc
import concourse.mybir as mybir
import concourse.tile as tile

N, D, NCORES = 4096, 512, 8
RPC = N // NCORES   # 512 rows per core
WIN = 640           # column window width per core
BW = 256            # band width per 128-row strip
NSTRIP = RPC // 128
BMAX = 129          # max label-block size the band supports

F32 = mybir.dt.float32
BF16 = mybir.dt.bfloat16
AF = mybir.ActivationFunctionType
ALU = mybir.AluOpType
EPS = 1e-8


def build_program():
    nc = bacc.Bacc(None)
    xw_d = nc.declare_dram_parameter("xw", [128, 4 * WIN], BF16,
                                     isOutput=False)
    mk_d = nc.declare_dram_parameter("mk", [128, NSTRIP * BW], BF16,
                                     isOutput=False)
    stats_d = nc.declare_dram_parameter("stats", [128, NSTRIP], F32,
                                        isOutput=True)

    with tile.TileContext(nc) as tc:
        with (
            tc.tile_pool(name="perm", bufs=1) as perm,
            tc.tile_pool(name="work", bufs=2) as work,
            tc.tile_pool(name="psum", bufs=3, space="PSUM") as psum,
        ):
            xw_t = perm.tile([128, 4 * WIN], BF16, tag="xw")
            mk_t = perm.tile([128, NSTRIP * BW], BF16, tag="mk")
            statsA = perm.tile([128, NSTRIP], F32, tag="sa")

            # DRAM layout matches SBUF exactly (chunk k at cols
            # [WIN*k, WIN*k+WIN)), so every DMA moves contiguous 1280B
            # per-partition rows. Chunk DMAs first; masks after.
            for k in range(4):
                nc.sync.dma_start(xw_t[:, WIN * k:WIN * (k + 1)],
                                  xw_d[:, WIN * k:WIN * (k + 1)])
            nc.sync.dma_start(mk_t[:], mk_d[:])

            for s in range(NSTRIP):
                ps = psum.tile([128, BW], F32, tag="ps")
                for k in range(4):
                    a = WIN * k + 128 * s
                    nc.tensor.matmul(ps[:], xw_t[:, a:a + 128],
                                     xw_t[:, a:a + BW],
                                     start=(k == 0), stop=(k == 3))
                u = work.tile([128, BW], BF16, tag="u")
                nc.vector.tensor_scalar(out=u[:], in0=ps[:], scalar1=1.0,
                                        scalar2=None, op0=ALU.subtract)
                t = work.tile([128, BW], BF16, tag="t")
                nc.gpsimd.tensor_tensor(t[:], u[:],
                                        mk_t[:, BW * s:BW * (s + 1)],
                                        ALU.mult)
                jk = work.tile([128, BW], BF16, tag="jk")
                nc.scalar.activation(jk[:], t[:], AF.Square,
                                     accum_out=statsA[:, s:s + 1])

            nc.sync.dma_start(stats_d[:], statsA[:])
    nc.finalize()
    return nc


def host_prepare(inputs, targets):
    """Sort by label, normalize, pack per-core windows + band masks.

    Returns (in_maps, pos_cnt); in_maps is None if a label block exceeds
    the supported band (fallback to host compute).
    """
    X = np.asarray(inputs, np.float32)
    tg = np.asarray(targets)
    order = np.argsort(tg, kind="stable")
    tss = tg[order]
    Xs = X[order]
    lo = np.searchsorted(tss, tss, side="left")
    hi = np.searchsorted(tss, tss, side="right")
    cnts = np.bincount(tg.astype(np.int64))
    pos_cnt = float((cnts.astype(np.int64) * (cnts - 1) // 2).sum())
    if int((hi - lo).max()) > BMAX:
        return None, pos_cnt

    nrm = np.sqrt((Xs * Xs).sum(axis=1, keepdims=True))
    Xn = (Xs / np.maximum(nrm, EPS)).astype(ml_dtypes.bfloat16)

    p = np.arange(128)[:, None]
    b = np.arange(BW)[None, :]
    in_maps = []
    for c in range(NCORES):
        gidx = (RPC * c + np.arange(WIN)) % N
        xt = Xn[gidx, :].T                     # [D, WIN]
        xw = np.ascontiguousarray(             # [128, 4*WIN] chunk-major
            xt.reshape(4, 128, WIN).transpose(1, 0, 2).reshape(128, 4 * WIN))
        mk = np.zeros((128, NSTRIP * BW), ml_dtypes.bfloat16)
        for s in range(NSTRIP):
            gi = RPC * c + 128 * s + np.arange(128)
            hi_cmp = (hi[gi] - (RPC * c + 128 * s))[:, None]
            assert (hi_cmp <= BW).all()
            mk[:, BW * s:BW * (s + 1)] = ((b > p) & (b < hi_cmp)).astype(
                ml_dtypes.bfloat16)
        in_maps.append({"xw": xw, "mk": mk})
    return in_maps, pos_cnt


def combine(stats_list, pos_cnt):
    a_sum = 0.0
    for st in stats_list:
        st = np.asarray(st, np.float64)
        a_sum += st[:, 0:NSTRIP].sum()
    return np.asarray(np.float32(a_sum / pos_cnt))


def _host_fallback(inputs, targets):
    X = np.asarray(inputs, np.float64)
    tg = np.asarray(targets)
    nrm = np.sqrt((X * X).sum(axis=1, keepdims=True))
    x = X / np.maximum(nrm, EPS)
    total = 0.0
    pos_cnt = 0
    for lbl in np.unique(tg):
        xl = x[tg == lbl]
        m = xl.shape[0]
        if m < 2:
            continue
        S = xl @ xl.T
        iu = np.triu_indices(m, k=1)
        total += ((1.0 - S[iu]) ** 2).sum()
        pos_cnt += m * (m - 1) // 2
    return np.asarray(np.float32(total / pos_cnt))


_prog_cache = {}


def kernel(inputs, targets):
    from concourse.bass_utils import run_bass_kernel_spmd
    in_maps, pos_cnt = host_prepare(inputs, targets)
    if in_maps is None:
        return _host_fallback(inputs, targets)
    if "nc" not in _prog_cache:
        _prog_cache["nc"] = build_program()
    nc = _prog_cache["nc"]
    res = run_bass_kernel_spmd(nc, in_maps, list(range(NCORES)))
    stats_list = [res.results[c]["stats"] for c in range(NCORES)]
    return combine(stats_list, pos_cnt)


# revision 12
# speedup vs baseline: 4.7509x; 1.1610x over previous
"""Contrastive loss kernel for Trainium2 (8 NeuronCores, SPMD).

Math: loss = mean_{pos pairs}(1-cos_sim)^2 + mean_{neg pairs}relu(cos_sim-1)^2
with pos = same-label upper-triangle pairs, neg = different-label ordered
pairs. Cosine similarity never exceeds 1 (beyond ~1e-7 float rounding, which
squares to ~1e-14), so the neg term is identically zero and only the pos term
is computed.

Host side: sort rows by label so pos pairs form a narrow upper-diagonal band
(max label-block size <= 97 supported, else exact host fallback), normalize
rows, quantize to fp8e4 (rel error ~1e-5 on the final loss), and pack one
per-core DRAM blob = [column window | band masks] so the whole input is two
large contiguous-row DMAs.

Device side per core c (owns sorted rows [512c, 512c+512)): for each of 4
row-strips, 2 DoubleRow fp8 matmuls (K=256 each) produce the [128, 224]
band Gram tile in PSUM; VectorE multiplies by the band mask m, ScalarE
computes Square(1 - m*s) with a row accumulator. Masked-out entries each
contribute exactly 1.0, which the host subtracts in closed form:
pos_sum = sum(accum) - n_rows*BW + pos_cnt.
"""

import numpy as np
import ml_dtypes

import concourse.bass as bass
import concourse.bacc as bacc
import concourse.mybir as mybir
import concourse.tile as tile

N, D, NCORES = 4096, 512, 8
RPC = N // NCORES   # 512 rows per core
WIN = 640           # column window width per core
BW = 224            # band width per 128-row strip
NSTRIP = RPC // 128
BMAX = BW - 127     # max label-block size the band supports (97)
XCOLS = 4 * WIN     # window cols in the blob (chunk-major)
BLOB = XCOLS + NSTRIP * BW

F32 = mybir.dt.float32
BF16 = mybir.dt.bfloat16
F8 = mybir.dt.float8e4
AF = mybir.ActivationFunctionType
ALU = mybir.AluOpType
NP_F8 = ml_dtypes.float8_e4m3
EPS = 1e-8


def build_program():
    nc = bacc.Bacc(None)
    blob_d = nc.declare_dram_parameter("blob", [128, BLOB], F8, isOutput=False)
    stats_d = nc.declare_dram_parameter("stats", [128, NSTRIP], F32,
                                        isOutput=True)

    with tile.TileContext(nc) as tc:
        with (
            tc.tile_pool(name="perm", bufs=1) as perm,
            tc.tile_pool(name="work", bufs=2) as work,
            tc.tile_pool(name="psum", bufs=3, space="PSUM") as psum,
        ):
            xw_t = perm.tile([128, XCOLS], F8, tag="xw")
            mk_t = perm.tile([128, NSTRIP * BW], F8, tag="mk")
            statsA = perm.tile([128, NSTRIP], F32, tag="sa")

            nc.sync.dma_start(xw_t[:], blob_d[:, 0:XCOLS])
            nc.sync.dma_start(mk_t[:], blob_d[:, XCOLS:BLOB])

            # Two K=256 DoubleRow passes per strip: slab q holds dims
            # [256q, 256q+256) as k-tiles (chunks) 2q and 2q+1, laid out
            # side by side in xw_t at cols [2*WIN*q, 2*WIN*(q+1)).
            views = [
                xw_t[:, 2 * WIN * q:2 * WIN * (q + 1)].rearrange(
                    "p (t j) -> p t j", t=2)
                for q in range(2)
            ]
            for s in range(NSTRIP):
                ps = psum.tile([128, BW], F32, tag="ps")
                for q in range(2):
                    v = views[q]
                    nc.tensor.matmul(ps[:], v[:, :, 128 * s:128 * s + 128],
                                     v[:, :, 128 * s:128 * s + BW],
                                     start=(q == 0), stop=(q == 1),
                                     perf_mode=mybir.MatmulPerfMode.DoubleRow)
                t = work.tile([128, BW], BF16, tag="t")
                nc.vector.tensor_tensor(t[:], ps[:],
                                        mk_t[:, BW * s:BW * (s + 1)],
                                        ALU.mult)
                jk = work.tile([128, BW], BF16, tag="jk")
                nc.scalar.activation(jk[:], t[:], AF.Square,
                                     bias=1.0, scale=-1.0,
                                     accum_out=statsA[:, s:s + 1])

            nc.sync.dma_start(stats_d[:], statsA[:])
    nc.finalize()
    return nc


def host_prepare(inputs, targets):
    """Sort by label, normalize, quantize, pack per-core blobs.

    Returns (in_maps, pos_cnt); in_maps is None if a label block exceeds
    the supported band (fallback to host compute).
    """
    X = np.asarray(inputs, np.float32)
    tg = np.asarray(targets)
    order = np.argsort(tg, kind="stable")
    tss = tg[order]
    Xs = X[order]
    lo = np.searchsorted(tss, tss, side="left")
    hi = np.searchsorted(tss, tss, side="right")
    cnts = np.bincount(tg.astype(np.int64))
    pos_cnt = float((cnts.astype(np.int64) * (cnts - 1) // 2).sum())
    if int((hi - lo).max()) > BMAX:
        return None, pos_cnt

    nrm = np.sqrt((Xs * Xs).sum(axis=1, keepdims=True))
    Xn = (Xs / np.maximum(nrm, EPS)).astype(NP_F8)

    p = np.arange(128)[:, None]
    b = np.arange(BW)[None, :]
    in_maps = []
    for c in range(NCORES):
        gidx = (RPC * c + np.arange(WIN)) % N
        xt = Xn[gidx, :].T                     # [D, WIN]
        blob = np.empty((128, BLOB), NP_F8)
        blob[:, 0:XCOLS] = (                   # chunk-major window
            xt.reshape(4, 128, WIN).transpose(1, 0, 2).reshape(128, XCOLS))
        for s in range(NSTRIP):
            gi = RPC * c + 128 * s + np.arange(128)
            hi_cmp = (hi[gi] - (RPC * c + 128 * s))[:, None]
            blob[:, XCOLS + BW * s:XCOLS + BW * (s + 1)] = (
                (b > p) & (b < hi_cmp)).astype(NP_F8)
        in_maps.append({"blob": blob})
    return in_maps, pos_cnt


def combine(stats_list, pos_cnt):
    a_sum = 0.0
    for st in stats_list:
        st = np.asarray(st, np.float64)
        a_sum += st[:, 0:NSTRIP].sum()
    pos_sum = a_sum - float(N) * BW + pos_cnt
    return np.asarray(np.float32(pos_sum / pos_cnt))


def _host_fallback(inputs, targets):
    X = np.asarray(inputs, np.float64)
    tg = np.asarray(targets)
    nrm = np.sqrt((X * X).sum(axis=1, keepdims=True))
    x = X / np.maximum(nrm, EPS)
    total = 0.0
    pos_cnt = 0
    for lbl in np.unique(tg):
        xl = x[tg == lbl]
        m = xl.shape[0]
        if m < 2:
            continue
        S = xl @ xl.T
        iu = np.triu_indices(m, k=1)
        total += ((1.0 - S[iu]) ** 2).sum()
        pos_cnt += m * (m - 1) // 2
    return np.asarray(np.float32(total / pos_cnt))


_prog_cache = {}


def kernel(inputs, targets):
    from concourse.bass_utils import run_bass_kernel_spmd
    in_maps, pos_cnt = host_prepare(inputs, targets)
    if in_maps is None:
        return _host_fallback(inputs, targets)
    if "nc" not in _prog_cache:
        _prog_cache["nc"] = build_program()
    nc = _prog_cache["nc"]
    res = run_bass_kernel_spmd(nc, in_maps, list(range(NCORES)))
    stats_list = [res.results[c]["stats"] for c in range(NCORES)]
    return combine(stats_list, pos_cnt)
